# revision 8
# baseline (speedup 1.0000x reference)
"""Trainium2 Bass kernel for nn_LDRFat (3-layer MLP forward).

reference: logits = relu((x @ W) @ fc_w.T + fc_b) @ logits_w.T + logits_b

Algebraic optimization: (x @ W) @ fc_w.T == x @ (W @ fc_w.T).
Precomputing Wfc = W @ fc_w.T ([3072,512]) collapses the dominant
309-GFLOP x@W matmul into a 51.5-GFLOP x@Wfc (phase B).

Device strategy (all matmul operands bf16, f32 accumulate):
 - Host feeds pre-transposed, pre-permuted operands laid out exactly as
   the SBUF tiles ([partition, free] contiguous): full-bandwidth DMAs,
   zero device transposes.
 - PE warm-up: ~30 throwaway matmuls issued at t=0 (inputs still in
   flight) so the HAM clock-gate reaches 2.4 GHz before phase A.
 - Phase A (sharded over W's k-rows, 3 k-tiles/core): wfc_shard =
   W[kshard,:] @ fc_w.T, nt-outer with 3 open PSUM groups, inputs
   DMA'd in 4 chunks so compute starts as soon as the first lands.
 - wfc AllGather split into two f-halves into separate tiles
   (wfc_lo/wfc_hi): phase B's ft 0/1 groups start after half 0 lands
   while half 1 is still on the wire. A tiny dummy AllGather at t=0
   absorbs collective first-use cost. Staging DMAs ride the scalar
   HWDGE queue so they never sit behind the 12.6 MB xT DMA on sync.
 - Phase B (data-parallel over batch, 2048 rows/core): h2T[f,m] =
   wfc-tiles (stationary) x xT (moving, free=512), ft-outer / kt / mc.
   ACT applies fused bias+relu. Logits computed as outT[cls,m] with
   logits_wT stationary, bias via a K=1 ones x logits_b matmul; host
   transposes the [10, 2048] per-core result back.
"""

import os
import numpy as np
import ml_dtypes

import concourse.bass as bass
import concourse.mybir as mybir
import concourse.tile as tile
from concourse import bacc
from concourse.bass import MemorySpace, ts, ds
from concourse.bass_utils import run_bass_kernel_spmd

B = 16384
N = 3072
FC = 512
CLS = 10
NCORES = 8
BS = B // NCORES     # 2048 batch rows per core
P = 128

KT = N // P          # 24 k/n tiles
FT = FC // P         # 4 f-tiles
MC = 4               # m-chunks per core
MCH = BS // MC       # 512
KSH = KT // NCORES   # 3 k-tiles per core in sharded precompute
WK = KSH * P         # 384 W-rows per core
NCHUNK = 4           # phase-A input DMA chunks
NTC = KT // NCHUNK   # nt-tiles per chunk
FH = FC // 2         # f-half for the split AllGather
NWARM = 30           # PE warm-up matmuls

F32 = mybir.dt.float32
BF16 = mybir.dt.bfloat16
BF = ml_dtypes.bfloat16

_CACHE = {}
LAST_RESULT = None


def build_kernel():
    nc = bacc.Bacc(
        "TRN2",
        target_bir_lowering=False,
        debug=False,
        enable_asserts=False,
        num_devices=NCORES,
    )
    # pre-permuted [partition, free] layouts (see prep_inputs)
    xT_d = nc.dram_tensor("xTr", [P, KT * BS], BF16, kind="ExternalInput").ap()
    wts_d = nc.dram_tensor("WTsr", [P, KT * WK], BF16, kind="ExternalInput").ap()
    fcwT_d = nc.dram_tensor("fcwTr", [P, KT * FC], BF16, kind="ExternalInput").ap()
    fcb_d = nc.dram_tensor("fc_b", [FC], F32, kind="ExternalInput").ap()
    lgwT_d = nc.dram_tensor("lgwT", [FC, CLS], BF16, kind="ExternalInput").ap()
    lgb_d = nc.dram_tensor("lgb", [CLS], BF16, kind="ExternalInput").ap()
    out_d = nc.dram_tensor("out", [CLS, BS], F32, kind="ExternalOutput").ap()

    with tile.TileContext(nc) as tc:
        with (
            tc.tile_pool(name="consts", bufs=1) as consts,
            tc.tile_pool(name="wfc", bufs=1) as wfc_pool,
            tc.tile_pool(name="xt", bufs=1) as xt_pool,
            tc.tile_pool(name="yt", bufs=1) as yt_pool,
            tc.tile_pool(name="osb", bufs=1) as out_pool,
        ):
            # ---- constants / small inputs (issued first on sync queue) ----
            fcb_sb = consts.tile([P, FT], F32)
            nc.sync.dma_start(fcb_sb, fcb_d.rearrange("(t p) -> p t", p=P))
            lgwT_sb = consts.tile([P, FT, CLS], BF16)
            nc.sync.dma_start(lgwT_sb, lgwT_d.rearrange("(t p) c -> p t c", p=P))
            lgb_sb = consts.tile([1, CLS], BF16)
            nc.sync.dma_start(lgb_sb, lgb_d.rearrange("(a c) -> a c", a=1))
            ones_stage = consts.tile([1, MCH], F32)
            nc.gpsimd.memset(ones_stage, 1.0)
            ones_sb = consts.tile([1, MCH], BF16)
            nc.vector.tensor_copy(ones_sb, ones_stage)

            # ---- bulk inputs: phase-A operands chunked first, then xT ----
            wts_sb = consts.tile([P, KT, WK], BF16)
            fcwT_sb = consts.tile([P, KT, FC], BF16)
            for c in range(NCHUNK):
                nc.sync.dma_start(
                    wts_sb[:, ts(c, NTC)].rearrange("p t k -> p (t k)"),
                    wts_d[:, ds(c * NTC * WK, NTC * WK)],
                )
                nc.sync.dma_start(
                    fcwT_sb[:, ts(c, NTC)].rearrange("p t f -> p (t f)"),
                    fcwT_d[:, ds(c * NTC * FC, NTC * FC)],
                )
            xt_sb = xt_pool.tile([P, KT, BS], BF16)
            nc.sync.dma_start(xt_sb.rearrange("p t m -> p (t m)"), xT_d)

            # wfc[k, f] split in two f-halves so phase B can start on half 0
            wfc_lo = wfc_pool.tile([P, KT, FH], BF16)
            wfc_hi = wfc_pool.tile([P, KT, FH], BF16)

            # ---------------- Phase A: wfc = W @ fc_w.T ----------------
            with (
                tc.tile_pool(name="wstg", bufs=1) as stage_pool,
                tc.tile_pool(name="ps_a", bufs=3, space=MemorySpace.PSUM) as ps_a,
                tc.tile_pool(name="ps_wu", bufs=1, space=MemorySpace.PSUM) as ps_wu,
                tc.tile_pool(name="ccd", bufs=1, space=MemorySpace.DRAM) as ccd,
            ):
                # PE warm-up while input DMAs are in flight
                wu_w = stage_pool.tile([P, P], BF16)
                nc.gpsimd.memset(wu_w, 0.0)
                wu_x = stage_pool.tile([P, MCH], BF16)
                nc.gpsimd.memset(wu_x, 0.0)
                wu_ps = ps_wu.tile([P, MCH], F32)
                for _ in range(NWARM):
                    nc.tensor.matmul(wu_ps, wu_w, wu_x, start=True, stop=True)

                # dummy collective to absorb ncfw first-use latency
                wu_g = stage_pool.tile([P, 16], BF16)
                nc.gpsimd.memset(wu_g, 0.0)
                gin_wu = ccd.tile([P, 16], BF16)
                nc.scalar.dma_start(gin_wu, wu_g)
                gout_wu = ccd.tile([NCORES * P, 16], BF16, addr_space="Shared")
                nc.gpsimd.collective_compute(
                    "AllGather",
                    mybir.AluOpType.bypass,
                    replica_groups=[list(range(NCORES))],
                    ins=[gin_wu.opt()],
                    outs=[gout_wu.opt()],
                )

                # h-major staging so each f-half is contiguous for its gather
                wfc_stage = stage_pool.tile([P, 2, KSH, FH], BF16)
                accs = [
                    ps_a.tile([P, FC], F32, tag="acc", name=f"acc{lkt}")
                    for lkt in range(KSH)
                ]
                for nt in range(KT):
                    for lkt in range(KSH):
                        nc.tensor.matmul(
                            accs[lkt], wts_sb[:, nt, ts(lkt, P)], fcwT_sb[:, nt],
                            start=(nt == 0), stop=(nt == KT - 1),
                        )
                for lkt in range(KSH):
                    for h in range(2):
                        nc.vector.tensor_copy(
                            wfc_stage[:, h, lkt], accs[lkt][:, ds(h * FH, FH)]
                        )

                # split AllGather: f-half 0 first (unblocks phase B ft 0/1),
                # half 1 pipelines underneath phase B
                for h, wfc_half in enumerate((wfc_lo, wfc_hi)):
                    gin = ccd.tile([P, KSH * FH], BF16, name=f"gin{h}")
                    nc.scalar.dma_start(
                        gin, wfc_stage[:, h].rearrange("p a b -> p (a b)")
                    )
                    gout = ccd.tile(
                        [NCORES * P, KSH * FH], BF16,
                        addr_space="Shared", name=f"gout{h}",
                    )
                    nc.gpsimd.collective_compute(
                        "AllGather",
                        mybir.AluOpType.bypass,
                        replica_groups=[list(range(NCORES))],
                        ins=[gin.opt()],
                        outs=[gout.opt()],
                    )
                    # gout rows = (core c, partition p); core c's shard is
                    # global k-tiles 3c..3c+2
                    nc.scalar.dma_start(
                        wfc_half.rearrange("p (c l) f -> p c (l f)", c=NCORES),
                        gout.rearrange("(c p) j -> p c j", p=P),
                    )

            # ------------ Phase B: h2T = relu(wfc.T @ xT + b) ------------
            with (
                tc.tile_pool(name="ps_b", bufs=6, space=MemorySpace.PSUM) as ps_b,
                tc.tile_pool(name="ps_lg", bufs=2, space=MemorySpace.PSUM) as ps_lg,
            ):
                out_sb = out_pool.tile([CLS, BS], F32)
                yts = []
                for ft in range(FT):
                    wfc_half = wfc_lo if ft < 2 else wfc_hi
                    fi = ft % 2
                    ps = [
                        ps_b.tile([P, MCH], F32, tag="h2", name=f"h2_{ft}_{mc}")
                        for mc in range(MC)
                    ]
                    for kt in range(KT):
                        for mc in range(MC):
                            nc.tensor.matmul(
                                ps[mc],
                                wfc_half[:, kt, ts(fi, P)],
                                xt_sb[:, kt, ts(mc, MCH)],
                                start=(kt == 0),
                                stop=(kt == KT - 1),
                            )
                    yt = yt_pool.tile([P, MC, MCH], BF16, tag=f"yt{ft}")
                    for mc in range(MC):
                        nc.scalar.activation(
                            yt[:, mc],
                            ps[mc],
                            mybir.ActivationFunctionType.Relu,
                            bias=fcb_sb[:, ds(ft, 1)],
                        )
                    yts.append(yt)

                # logits: outT[cls, m] per m-chunk; stationary = lgwT tiles
                for mc in range(MC):
                    plg = ps_lg.tile([CLS, MCH], F32, tag="lg")
                    for ft in range(FT):
                        nc.tensor.matmul(
                            plg,
                            lgwT_sb[:, ft],
                            yts[ft][:, mc],
                            start=(ft == 0),
                            stop=False,
                        )
                    nc.tensor.matmul(plg, lgb_sb, ones_sb, start=False, stop=True)
                    nc.vector.tensor_copy(out_sb[:, ts(mc, MCH)], plg)

                nc.sync.dma_start(out_d, out_sb)

    nc.compile()
    return nc


def _permute(a2d, rows_per_tile=P):
    """[T*P, F] -> [P, T*F] so partition p's data is contiguous in DRAM."""
    t = a2d.shape[0] // rows_per_tile
    return np.ascontiguousarray(
        a2d.reshape(t, rows_per_tile, a2d.shape[1])
        .transpose(1, 0, 2)
        .reshape(rows_per_tile, t * a2d.shape[1])
    )


def prep_inputs(inputs):
    """Host-side layout marshaling: slice per core, pre-transpose, bf16."""
    x = np.asarray(inputs["x"], dtype=np.float32)
    W = np.asarray(inputs["W"], dtype=np.float32)
    fc_w = np.asarray(inputs["fc_w"], dtype=np.float32)
    fc_b = np.ascontiguousarray(inputs["fc_b"], dtype=np.float32)
    lgw = np.asarray(inputs["logits_w"], dtype=np.float32)
    lgb = np.asarray(inputs["logits_b"], dtype=np.float32)

    xT = x.astype(BF).T                              # [N, B] view
    WT = W.astype(BF).T                              # [N, N] rows=n, cols=k
    fcwTr = _permute(np.ascontiguousarray(fc_w.astype(BF).T))  # [P, KT*FC]
    lgwT = np.ascontiguousarray(lgw.astype(BF).T)    # [FC, CLS]
    lgb_bf = lgb.astype(BF)

    in_maps = []
    for i in range(NCORES):
        m = {
            "xTr": _permute(np.ascontiguousarray(xT[:, i * BS : (i + 1) * BS])),
            "WTsr": _permute(np.ascontiguousarray(WT[:, i * WK : (i + 1) * WK])),
            "fcwTr": fcwTr,
            "fc_b": fc_b,
            "lgwT": lgwT,
            "lgb": lgb_bf,
        }
        in_maps.append(m)
    return in_maps


def kernel(**inputs) -> np.ndarray:
    global LAST_RESULT
    if "nc" not in _CACHE:
        _CACHE["nc"] = build_kernel()
    nc = _CACHE["nc"]

    in_maps = prep_inputs(inputs)
    res = run_bass_kernel_spmd(
        nc,
        in_maps,
        core_ids=list(range(NCORES)),
        trace=bool(int(os.environ.get("KERNEL_TRACE", "0"))),
    )
    LAST_RESULT = res
    # per-core out is [CLS, BS]; transpose back to [BS, CLS]
    out = np.concatenate(
        [np.ascontiguousarray(r_["out"].T) for r_ in res.results], axis=0
    )
    return out
